# revision 16
# baseline (speedup 1.0000x reference)
"""DigitCapsules routing kernel for 8 Trainium2 NeuronCores.

Math: in the reference, u_hat is an explicit broadcast of u_core over the
capsule axis i, so b stays constant along i in every routing iteration,
softmax over i is exactly uniform (1/K), and the whole 3-iteration routing
collapses (exactly, in floating point too) to:

    v[b, i, :] = squash((1/576) * sum_{r,k} x2[b, r, k] * W[b, r, k, :])

broadcast over i = 0..575, where x2 = x.reshape(B, 8, 576).transpose(0, 2, 1).

Sharding: batch dim B=32 across 8 cores, 4 batches per core (data parallel,
matching the hint).  Per core the kernel contracts over (r, k) = 4608 with
TensorE (x-tile stationary, W moving, fp32 PSUM accumulation), extracts the
k-diagonal via an affine-select mask + one-hot matmul, applies squash
on-chip, and writes the i-broadcast output with one 0-stride-source DMA.

Walrus allows only ONE sync wait per Matmult/DMACopy/TensorTensor, so the
structure keeps every instruction at <=1 cross-proc wait: the host packs x2
as 8 extra trailing columns of the W rows and pads the route dim to 640
(wx = [W | x2] -> [NB, 640, 136], zero rows are harmless in the
contraction), giving ONE input DMA per batch with no lane sharing; constants
are built on-chip (memset + affine_select on DVE); the single output DMA is
emitted first so it owns HWDGE lane 0 and waits only on the DVE semaphore.
"""

import numpy as np

import concourse.bass as bass
import concourse.bacc as bacc
import concourse.mybir as mybir
import concourse.tile as tile
from concourse.bass_utils import run_bass_kernel_spmd

N_CORES = 8
B, C, H, W_ = 32, 8, 24, 24
R = H * W_          # 576 routes
RP = 640            # padded routes (5 tiles of 128)
KJ = 128            # fused (k=8, j=16) axis
D = 16
NB = B // N_CORES   # 4 batches per core
NTILE = RP // 128   # 5
WX = KJ + C         # 136 = W row + packed x2 row

_cached_nc = None
_last_in_maps = None


def _build():
    # Bacc (not raw Bass): its compile() pass splits sync waits into event
    # semaphores to satisfy the TRN2 one-wait-per-instruction constraint.
    nc = bacc.Bacc(trn_type="TRN2")
    f32 = mybir.dt.float32

    wx_h = nc.dram_tensor("wx", [NB, RP, WX], f32, kind="ExternalInput")
    out_h = nc.dram_tensor("out", [NB, R, D], f32, kind="ExternalOutput")

    with tile.TileContext(nc) as tc:
        with (
            tc.tile_pool(name="consts", bufs=1) as consts,
            tc.tile_pool(name="wp", bufs=NB) as wp,
            tc.tile_pool(name="gps", bufs=NB, space="PSUM") as gps,
            tc.tile_pool(name="taps", bufs=1, space="PSUM") as taps,
            tc.tile_pool(name="sm", bufs=24) as sm,
        ):
            # mask[k, k'*16+j] = (k == k'): selects the k-diagonal of G.
            # Built on gpsimd (affine_select lives there), then hopped
            # through a DVE copy so downstream DVE consumers stay at one
            # cross-proc wait.
            mask_raw = consts.tile([8, KJ], f32)
            nc.gpsimd.memset(mask_raw[:], 1.0)
            nc.gpsimd.affine_select(
                out=mask_raw[:], in_=mask_raw[:],
                compare_op=mybir.AluOpType.is_equal, fill=0.0,
                base=0, pattern=[[1, 8], [0, 16]], channel_multiplier=-1,
            )
            mask_t = consts.tile([8, KJ], f32)
            nc.vector.tensor_copy(mask_t[:], mask_raw[:])
            # oneh[:, n*4+m] = (m == n): column-sums pm[n] into row n of TA.
            oneh_t = consts.tile([8, 4 * NB], f32)
            nc.vector.memset(oneh_t[:], 0.0)
            for n in range(NB):
                nc.vector.memset(oneh_t[:, n * 5:n * 5 + 1], 1.0)

            # G[n][k, k'*16+j] = sum_r x2[n, r, k] * W[n, r, k'*16+j]
            pms = []
            for n in range(NB):
                wx_t = wp.tile([128, NTILE, WX], f32)
                nc.sync.dma_start(
                    wx_t[:], wx_h[n].rearrange("(d p) f -> p d f", p=128)
                )
                g = gps.tile([8, KJ], f32)
                for d in range(NTILE):
                    nc.tensor.matmul(
                        g[:], wx_t[:, d, KJ:WX], wx_t[:, d, :KJ],
                        start=(d == 0), stop=(d == NTILE - 1),
                    )
                pm = sm.tile([8, KJ], f32)
                nc.vector.tensor_mul(pm[:], g[:], mask_t[:])
                pms.append(pm)

            # TA[n, k'*16+j] = sum_k pm[n][k, k'*16+j]  (column sums, row n)
            ta = taps.tile([NB, KJ], f32)
            for n in range(NB):
                nc.tensor.matmul(
                    ta[:], oneh_t[:, n * 4:(n + 1) * 4], pms[n][:],
                    start=(n == 0), stop=(n == NB - 1),
                )

            # T[n, j] = sum_k' TA[n, k'*16+j]
            t_t = sm.tile([NB, D], f32)
            nc.vector.reduce_sum(
                t_t[:], ta[:].rearrange("p (k j) -> p j k", j=D),
                axis=mybir.AxisListType.X,
            )

            # squash(s), s = T/576: v = s * norm/((1+norm)*sqrt(norm+1e-8))
            s = sm.tile([NB, D], f32)
            nc.vector.tensor_scalar_mul(s[:], t_t[:], 1.0 / float(R))
            sq = sm.tile([NB, D], f32)
            nc.vector.tensor_mul(sq[:], s[:], s[:])
            norm = sm.tile([NB, 1], f32)
            nc.vector.reduce_sum(norm[:], sq[:], axis=mybir.AxisListType.X)
            norm_eps = sm.tile([NB, 1], f32)
            nc.vector.tensor_scalar_add(norm_eps[:], norm[:], 1e-8)
            sqn = sm.tile([NB, 1], f32)
            nc.scalar.sqrt(sqn[:], norm_eps[:])
            np1 = sm.tile([NB, 1], f32)
            nc.vector.tensor_scalar_add(np1[:], norm[:], 1.0)
            den = sm.tile([NB, 1], f32)
            nc.vector.tensor_mul(den[:], np1[:], sqn[:])
            rec = sm.tile([NB, 1], f32)
            nc.vector.reciprocal(rec[:], den[:])
            coef = sm.tile([NB, 1], f32)
            nc.vector.tensor_mul(coef[:], norm[:], rec[:])
            v = sm.tile([NB, D], f32)
            nc.vector.tensor_scalar_mul(v[:], s[:], coef[:])

            # out[n, i, :] = v[n, :] for all i — 0-stride-source broadcast
            src = v[:, :].unsqueeze(1).broadcast_to([NB, R, D])
            nc.sync.dma_start(out_h[:, :, :], src)

    # Bacc defers register allocation to finalize(); run_bass_via_pjrt
    # serializes a prebuilt module without finalizing it.
    nc.finalize()
    return nc


def kernel(x, route_weights):
    global _cached_nc, _last_in_maps
    if _cached_nc is None:
        _cached_nc = _build()
    nc = _cached_nc

    x = np.ascontiguousarray(np.asarray(x), dtype=np.float32)
    w = np.ascontiguousarray(np.asarray(route_weights), dtype=np.float32)
    x2 = x.reshape(B, C, R).transpose(0, 2, 1)          # [B, R, 8]
    wf = w.reshape(B, R, KJ)
    wx = np.zeros((B, RP, WX), np.float32)
    wx[:, :R, :KJ] = wf
    wx[:, :R, KJ:] = x2

    in_maps = [
        {"wx": np.ascontiguousarray(wx[c * NB:(c + 1) * NB])}
        for c in range(N_CORES)
    ]
    _last_in_maps = in_maps

    res = run_bass_kernel_spmd(nc, in_maps, core_ids=list(range(N_CORES)))
    return np.concatenate([r["out"] for r in res.results], axis=0)
